# revision 10
# baseline (speedup 1.0000x reference)
"""Fully-connected GNN message-passing kernel for Trainium2 (8 NeuronCores).

Strategy
--------
The reference graph is fully connected (each graph: all ordered pairs i != j).
This lets us replace gather/segment_sum with dense per-graph math:

  edge-MLP layer 1:  concat([x[i], x[j]]) @ we1 == a_i + b_j
      with a = x @ we1[:H], b = x @ we1[H:]          (tiny matmuls)
  messages for ALL i,j pairs (incl. diagonal) are computed densely;
  agg_i = sum_j silu(silu(a_i+b_j+be1) @ we2 + be2) - diagonal_term_i

Sharding: data-parallel over graphs, 2 graphs per core, weights replicated.
All feature-major ("transposed") layouts on-chip: features on partitions,
nodes/edges along the free dimension.

Engine assignment (per core):
  PE   - all matmuls (edge MLP2, a/b, node MLP, embeddings)
  ACT  - the two per-edge SiLU stages (the roofline: 1 elem/lane/cycle)
  DVE  - broadcast-build of a_i+b_j, per-receiver reductions, residuals
  DMA  - input/weight loads, output store
"""

import numpy as np

# problem shapes (hardcoded per contract)
BS, N, IN_NF, H, EH, OUT_NF, L = 16, 128, 64, 256, 128, 64, 4
NCORES = 8
GPC = BS // NCORES            # graphs per core
NODES = GPC * N               # nodes per core
HC = H // 128                 # H partition chunks

# tuning knobs
SLAB_RECV = 8                 # receivers per edge slab
SLAB = SLAB_RECV * N          # slab columns (edges)
NSLABS = N // SLAB_RECV       # slabs per graph
MMQ = 512                     # matmul moving-dim slice (fp32 limit)

Z1_MODE = "pe"
MM_DT = "bf16"

_CACHE = {}


def _silu_np(x):
    return x / (1.0 + np.exp(-x))


def _canonical_edges():
    r = np.repeat(np.arange(N), N)
    c = np.tile(np.arange(N), N)
    m = r != c
    r, c = r[m], c[m]
    off = (np.arange(BS) * N)[:, None]
    rows = (r[None, :] + off).reshape(-1)
    cols = (c[None, :] + off).reshape(-1)
    return rows, cols


def _edges_match(rows, cols):
    """True if (rows, cols) describe the canonical fully-connected batch
    (any edge ordering that keeps each graph's edge block内 self-contained
    is fine for segment_sum, but we require the exact multiset of edges)."""
    er, ec = _canonical_edges()
    rows = np.asarray(rows).astype(np.int64).ravel()
    cols = np.asarray(cols).astype(np.int64).ravel()
    if rows.shape != er.shape or cols.shape != ec.shape:
        return False
    if np.array_equal(rows, er) and np.array_equal(cols, ec):
        return True
    # permuted edge list: compare sorted edge keys
    k1 = np.sort(rows * (BS * N) + cols)
    k2 = np.sort(er * (BS * N) + ec)
    return np.array_equal(k1, k2)


def _numpy_reference(h, rows, cols, w_in, b_in, w_out, b_out,
                     we1, be1, we2, be2, wn1, bn1, wn2, bn2):
    """Exact fallback (only used if inputs are not the canonical FC batch)."""
    f = np.float32
    x = h.reshape(BS * N, -1).astype(f) @ w_in.astype(f) + b_in.astype(f)
    rows = np.asarray(rows).astype(np.int64)
    cols = np.asarray(cols).astype(np.int64)
    for l in range(L):
        m = np.concatenate([x[rows], x[cols]], axis=-1)
        m = _silu_np(m @ we1[l].astype(f) + be1[l].astype(f))
        m = _silu_np(m @ we2[l].astype(f) + be2[l].astype(f))
        agg = np.zeros((BS * N, m.shape[-1]), f)
        np.add.at(agg, rows, m)
        u = np.concatenate([x, agg], axis=-1)
        u = _silu_np(u @ wn1[l].astype(f) + bn1[l].astype(f))
        u = u @ wn2[l].astype(f) + bn2[l].astype(f)
        x = x + u
    return x @ w_out.astype(f) + b_out.astype(f)


def _split_excess_waits(nc, mybir, cap=1):
    """The walrus build in this environment accepts only one sync-wait per
    instruction; move extra waits onto preceding same-engine NOPs."""
    n_split = 0
    for fn in nc.m.functions:
        for blk in fn.blocks:
            il = blk.instructions
            new = []
            changed = False
            for ins in il:
                si = ins.sync_info
                if si is not None and si.on_wait and len(si.on_wait) > cap:
                    waits = list(si.on_wait)
                    keep, extra = waits[-cap:], waits[:-cap]
                    for w in extra:
                        nop = mybir.InstNoOp(name=f"I-wsplit-{nc.next_id()}",
                                             ins=[], outs=[])
                        nop.engine = ins.engine
                        nop.sync_info = mybir.SyncInfo(on_wait=[w], on_update=[])
                        new.append(nop)
                        n_split += 1
                    ins.sync_info = mybir.SyncInfo(on_wait=keep,
                                                   on_update=list(si.on_update))
                    changed = True
                new.append(ins)
            if changed:
                il[:] = new
    return n_split


def _build_nc(split_waits=True):
    import concourse.bass as bass
    import concourse.tile as tile
    import concourse.mybir as mybir
    from contextlib import ExitStack

    f32 = mybir.dt.float32
    bf16 = mybir.dt.bfloat16
    mdt = f32 if MM_DT == "f32" else bf16
    AF = mybir.ActivationFunctionType
    ALU = mybir.AluOpType

    nc = bass.Bass()

    # ---- DRAM parameters (per core) ----
    h_d = nc.declare_dram_parameter("h_c", [NODES, IN_NF], f32, isOutput=False)
    w_in_d = nc.declare_dram_parameter("w_in", [IN_NF, H], mdt, isOutput=False)
    b_in_d = nc.declare_dram_parameter("b_in", [H], f32, isOutput=False)
    w_out_d = nc.declare_dram_parameter("w_out", [H, OUT_NF], f32, isOutput=False)
    b_out_d = nc.declare_dram_parameter("b_out", [OUT_NF], f32, isOutput=False)
    we1_d = nc.declare_dram_parameter("we1", [L, 2 * H, EH], mdt, isOutput=False)
    be1_d = nc.declare_dram_parameter("be1", [L, EH], f32, isOutput=False)
    we2_d = nc.declare_dram_parameter("we2", [L, EH, EH], mdt, isOutput=False)
    be2_d = nc.declare_dram_parameter("be2", [L, EH], f32, isOutput=False)
    wn1_d = nc.declare_dram_parameter("wn1", [L, H + EH, H], mdt, isOutput=False)
    bn1_d = nc.declare_dram_parameter("bn1", [L, H], f32, isOutput=False)
    wn2_d = nc.declare_dram_parameter("wn2", [L, H, H], mdt, isOutput=False)
    bn2_d = nc.declare_dram_parameter("bn2", [L, H], f32, isOutput=False)
    ident_d = nc.declare_dram_parameter("ident", [128, 128], mdt, isOutput=False)
    identf_d = nc.declare_dram_parameter("identf", [128, 128], f32, isOutput=False)
    ones_d = nc.declare_dram_parameter("ones_r", [1, 128], f32, isOutput=False)
    out_d = nc.declare_dram_parameter("out_c", [NODES, OUT_NF], f32, isOutput=True)

    with tile.TileContext(nc) as tc, ExitStack() as ctx:
        consts = ctx.enter_context(tc.tile_pool(name="consts", bufs=1))
        work = ctx.enter_context(tc.tile_pool(name="work", bufs=2))
        mpool = ctx.enter_context(tc.tile_pool(name="mp", bufs=2))
        xpool = ctx.enter_context(tc.tile_pool(name="xp", bufs=2))
        psum = ctx.enter_context(tc.tile_pool(name="ps", bufs=2, space="PSUM"))

        dma = nc.sync.dma_start

        # warm the ACT Silu table immediately (zero-dependency dummy op)
        warm = work.tile([1, 2], f32, tag="warm", name="warm")
        nc.vector.memset(warm[0:1, 0:1], 0.0)
        nc.scalar.activation(warm[0:1, 1:2], warm[0:1, 0:1], AF.Silu)

        # ---- input loads (h first: it heads the critical path) ----
        hns = []
        for nb in range(NODES // 128):
            hn = work.tile([128, IN_NF], f32, tag="hn", name=f"hn_{nb}")
            dma(out=hn[:], in_=h_d[nb * 128:(nb + 1) * 128, :])
            hns.append(hn)

        # ---- constant loads ----
        identf_sb = consts.tile([128, 128], f32, tag="identf", name="identf_sb")
        dma(out=identf_sb[:], in_=identf_d[:])
        w_in_sb = consts.tile([IN_NF, H], mdt, tag="w_in", name="w_in_sb")
        dma(out=w_in_sb[:], in_=w_in_d[:])
        b_in_sb = consts.tile([128, HC], f32, tag="b_in", name="b_in_sb")
        dma(out=b_in_sb[:], in_=b_in_d.rearrange("(m p) -> p m", p=128))

        ident_sb = None
        if Z1_MODE == "pe":
            if MM_DT == "f32":
                ident_sb = identf_sb
            else:
                ident_sb = consts.tile([128, 128], mdt, tag="ident", name="ident_sb")
                dma(out=ident_sb[:], in_=ident_d[:])

        # ---- input embedding: x_T[m] = (h @ w_in + b_in)^T ----
        hT = work.tile([IN_NF, NODES], mdt, tag="hT", name="hT")
        for nb in range(NODES // 128):
            hTp = psum.tile([IN_NF, 128], f32, tag=f"zp{nb}", bufs=1, name=f"hTp_{nb}")
            nc.tensor.transpose(hTp[:], hns[nb][:], identf_sb[:])
            nc.vector.tensor_copy(hT[:, nb * 128:(nb + 1) * 128], hTp[:])

        x_T = [xpool.tile([128, NODES], f32, tag=f"x{m}", name=f"x0_{m}")
               for m in range(HC)]
        xb = [xpool.tile([128, NODES], mdt, tag=f"xb{m}", name=f"xb0_{m}")
              for m in range(HC)]
        for g in range(GPC):
            gb = slice(g * N, (g + 1) * N)
            for m in range(HC):
                xp_ = psum.tile([128, N], f32, tag=f"mp{m}", bufs=1,
                                name=f"xemb_{g}_{m}")
                nc.tensor.matmul(xp_[:], lhsT=w_in_sb[:, m * 128:(m + 1) * 128],
                                 rhs=hT[:, gb], start=True, stop=True)
                nc.vector.tensor_scalar_add(x_T[m][:, gb], xp_[:], b_in_sb[:, m:m + 1])
                nc.vector.tensor_scalar_add(xb[m][:, gb], xp_[:], b_in_sb[:, m:m + 1])

        # ---- layers ----
        we1_sb, we2_sb, wn1_sb, wn2_sb = [], [], [], []
        be1_sb = consts.tile([EH, L], f32, tag="be1", name="be1_sb")
        dma(out=be1_sb[:], in_=be1_d.rearrange("l p -> p l"))
        be2_sb = consts.tile([EH, L], f32, tag="be2", name="be2_sb")
        dma(out=be2_sb[:], in_=be2_d.rearrange("l p -> p l"))
        bn1_sb = consts.tile([128, L * HC], f32, tag="bn1", name="bn1_sb")
        dma(out=bn1_sb[:], in_=bn1_d.rearrange("l (m p) -> p (l m)", p=128))
        bn2_sb = consts.tile([128, L * HC], f32, tag="bn2", name="bn2_sb")
        dma(out=bn2_sb[:], in_=bn2_d.rearrange("l (m p) -> p (l m)", p=128))
        for l in range(L):
            t1 = []
            for j in range(4):
                t = consts.tile([128, EH], mdt, tag=f"we1_{l}_{j}", name=f"we1_{l}_{j}")
                dma(out=t[:], in_=we1_d[l, j * 128:(j + 1) * 128, :])
                t1.append(t)
            we1_sb.append(t1)
            t = consts.tile([EH, EH], mdt, tag=f"we2_{l}", name=f"we2_{l}")
            dma(out=t[:], in_=we2_d[l])
            we2_sb.append(t)
            tn = []
            for k in range(3):
                t = consts.tile([128, H], mdt, tag=f"wn1_{l}_{k}", name=f"wn1_{l}_{k}")
                dma(out=t[:], in_=wn1_d[l, k * 128:(k + 1) * 128, :])
                tn.append(t)
            wn1_sb.append(tn)
            tn = []
            for k in range(2):
                t = consts.tile([128, H], mdt, tag=f"wn2_{l}_{k}", name=f"wn2_{l}_{k}")
                dma(out=t[:], in_=wn2_d[l, k * 128:(k + 1) * 128, :])
                tn.append(t)
            wn2_sb.append(tn)
        w_out_sb = []
        for k in range(HC):
            t = consts.tile([128, OUT_NF], f32, tag=f"w_out_{k}", name=f"w_out_{k}")
            dma(out=t[:], in_=w_out_d[k * 128:(k + 1) * 128, :])
            w_out_sb.append(t)
        b_out_sb = consts.tile([1, OUT_NF], f32, tag="b_out", name="b_out_sb")
        dma(out=b_out_sb[:], in_=b_out_d[:].unsqueeze(0))
        ones_sb = consts.tile([1, 128], f32, tag="ones", name="ones_sb")
        dma(out=ones_sb[:], in_=ones_d[:])

        RPQ = MMQ // N                # receivers per matmul slice
        for l in range(L):
            # a, b projections (feature-major), per graph so the next layer's
            # prep for graph 0 can overlap this layer's graph-1 tail.
            abTm = work.tile([EH, NODES], mdt, tag="abTm", name=f"abTm_{l}")
            bTm = work.tile([EH, NODES], mdt, tag="bTm", name=f"bTm_{l}")
            aggT = work.tile([EH, NODES], f32, tag="agg", name=f"agg_{l}")
            diagT = work.tile([EH, NODES], f32, tag="diag", name=f"diag_{l}")
            b4 = [None] * GPC
            for g in range(GPC):
                gb = slice(g * N, (g + 1) * N)
                ap_ = psum.tile([EH, N], f32, tag=f"zp{g}", bufs=1,
                                name=f"ap_{l}_{g}")
                for k in range(HC):
                    nc.tensor.matmul(ap_[:], lhsT=we1_sb[l][k][:],
                                     rhs=xb[k][:, gb],
                                     start=(k == 0), stop=(k == HC - 1))
                nc.vector.tensor_scalar_add(abTm[:, gb], ap_[:], be1_sb[:, l:l + 1])
                bp_ = psum.tile([EH, N], f32, tag=f"mp{g}", bufs=1,
                                name=f"bp_{l}_{g}")
                for k in range(HC):
                    nc.tensor.matmul(bp_[:], lhsT=we1_sb[l][HC + k][:],
                                     rhs=xb[k][:, gb],
                                     start=(k == 0), stop=(k == HC - 1))
                # materialize [b, b, .., b] (RPQ copies) straight from psum
                # so the B-term of the z1 broadcast runs as one N=MMQ matmul
                b4[g] = work.tile([EH, MMQ], mdt, tag=f"b4_{g}", name=f"b4_{l}_{g}")
                nc.scalar.activation(
                    b4[g][:].rearrange("p (r j) -> p r j", r=RPQ),
                    bp_[:].unsqueeze(1).broadcast_to([EH, RPQ, N]), AF.Copy)
                if Z1_MODE != "pe":
                    nc.vector.tensor_copy(bTm[:, gb], bp_[:])

            # edge slabs — interleave the two graphs as independent pipelines
            # (separate psum tags) so ACT always has a ready op while the
            # other chain's matmuls are in flight.
            SKEW = 4   # g0 runs ahead so g1's tail covers g0's boundary chain
            order = [(s, 0) for s in range(SKEW)]
            for s in range(NSLABS):
                order.append((s, 1))
                if s + SKEW < NSLABS:
                    order.append((s + SKEW, 0))
            for s, g in order:
                if True:
                    r0 = s * SLAB_RECV
                    if Z1_MODE == "pe":
                        zp = psum.tile([EH, SLAB], f32, tag=f"zp{g}", bufs=1,
                                       name=f"zp_{l}_{g}_{s}")
                        for q in range(SLAB // MMQ):
                            base = g * N + r0 + RPQ * q
                            rhs_a = abTm[:, base: base + RPQ] \
                                .unsqueeze(2).broadcast_to([EH, RPQ, N])
                            nc.tensor.matmul(zp[:, q * MMQ:(q + 1) * MMQ],
                                             lhsT=ident_sb[:], rhs=rhs_a,
                                             start=True, stop=False)
                            nc.tensor.matmul(zp[:, q * MMQ:(q + 1) * MMQ],
                                             lhsT=ident_sb[:], rhs=b4[g][:],
                                             start=False, stop=True)
                        z1 = zp
                    else:
                        z1 = mpool.tile([EH, SLAB], f32, tag="z1", name=f"z1_{l}_{g}_{s}")
                        for r in range(SLAB_RECV):
                            col = g * N + r0 + r
                            nc.vector.tensor_scalar_add(
                                z1[:, r * N:(r + 1) * N],
                                bTm[:, g * N:(g + 1) * N],
                                abTm[:, col:col + 1])
                    m1 = mpool.tile([EH, SLAB], mdt, tag=f"m1_{g}", bufs=3,
                                    name=f"m1_{l}_{g}_{s}")
                    nc.scalar.activation(m1[:], z1[:], AF.Silu)
                    mp_ = psum.tile([EH, SLAB], f32, tag=f"mp{g}", bufs=1,
                                    name=f"mp_{l}_{g}_{s}")
                    for q in range(SLAB // MMQ):
                        nc.tensor.matmul(mp_[:, q * MMQ:(q + 1) * MMQ],
                                         lhsT=we2_sb[l][:],
                                         rhs=m1[:, q * MMQ:(q + 1) * MMQ],
                                         start=True, stop=True)
                    m2 = mpool.tile([EH, SLAB], f32, tag=f"m2_{g}", bufs=3,
                                    name=f"m2_{l}_{g}_{s}")
                    nc.scalar.activation(m2[:], mp_[:], AF.Silu, bias=be2_sb[:, l:l + 1])
                    red_in = m2[:].rearrange("p (i j) -> p i j", j=N)
                    nc.vector.tensor_reduce(
                        aggT[:, g * N + r0: g * N + r0 + SLAB_RECV],
                        red_in, axis=mybir.AxisListType.X, op=ALU.add)
                    # pull out the diagonal (j == i) messages for correction:
                    # within this slab, receiver r0+r's own column is r*N+r0+r
                    diag_ap = bass.AP(
                        tensor=m2.tensor, offset=m2.offset + r0,
                        ap=[m2.ap[0], [N + 1, SLAB_RECV]])
                    nc.vector.tensor_copy(
                        diagT[:, g * N + r0: g * N + r0 + SLAB_RECV], diag_ap)

            # per-graph: subtract diagonal, node MLP, residual.
            u1 = [work.tile([128, NODES], mdt, tag=f"u1_{m}", name=f"u1_{l}_{m}")
                  for m in range(HC)]
            x_new = [xpool.tile([128, NODES], f32, tag=f"x{m}", name=f"x{l + 1}_{m}")
                     for m in range(HC)]
            xb_new = [xpool.tile([128, NODES], mdt, tag=f"xb{m}",
                                 name=f"xb{l + 1}_{m}") for m in range(HC)]
            for g in range(GPC):
                gb = slice(g * N, (g + 1) * N)
                nc.vector.tensor_sub(aggT[:, gb], aggT[:, gb], diagT[:, gb])
                aggb = work.tile([EH, N], mdt, tag=f"aggb{g}",
                                 name=f"aggb_{l}_{g}")
                nc.vector.tensor_copy(aggb[:], aggT[:, gb])

                for m in range(HC):
                    tag = f"zp{g}" if m == 0 else f"mp{g}"
                    up = psum.tile([128, N], f32, tag=tag, bufs=1,
                                   name=f"u1p_{l}_{g}_{m}")
                    nc.tensor.matmul(up[:], lhsT=wn1_sb[l][0][:, m * 128:(m + 1) * 128],
                                     rhs=xb[0][:, gb], start=True, stop=False)
                    nc.tensor.matmul(up[:], lhsT=wn1_sb[l][1][:, m * 128:(m + 1) * 128],
                                     rhs=xb[1][:, gb], start=False, stop=False)
                    nc.tensor.matmul(up[:], lhsT=wn1_sb[l][2][:, m * 128:(m + 1) * 128],
                                     rhs=aggb[:], start=False, stop=True)
                    nc.scalar.activation(u1[m][:, gb], up[:], AF.Silu,
                                         bias=bn1_sb[:, l * HC + m: l * HC + m + 1])
                for m in range(HC):
                    tag = f"zp{g}" if m == 0 else f"mp{g}"
                    u2p = psum.tile([128, N], f32, tag=tag, bufs=1,
                                    name=f"u2p_{l}_{g}_{m}")
                    nc.tensor.matmul(u2p[:], lhsT=wn2_sb[l][0][:, m * 128:(m + 1) * 128],
                                     rhs=u1[0][:, gb], start=True, stop=False)
                    nc.tensor.matmul(u2p[:], lhsT=wn2_sb[l][1][:, m * 128:(m + 1) * 128],
                                     rhs=u1[1][:, gb], start=False, stop=True)
                    nc.vector.scalar_tensor_tensor(
                        x_new[m][:, gb], u2p[:],
                        bn2_sb[:, l * HC + m: l * HC + m + 1], x_T[m][:, gb],
                        op0=ALU.add, op1=ALU.add)
                    nc.vector.tensor_copy(xb_new[m][:, gb], x_new[m][:, gb])
            x_T = x_new
            xb = xb_new

        # ---- output embedding: out = x @ w_out + b_out (natural layout) ----
        for nb in range(NODES // 128):
            op_ = psum.tile([128, OUT_NF], f32, tag=f"zp{nb}", bufs=1, name=f"outp_{nb}")
            nc.tensor.matmul(op_[:], lhsT=x_T[0][:, nb * 128:(nb + 1) * 128],
                             rhs=w_out_sb[0][:], start=True, stop=False)
            nc.tensor.matmul(op_[:], lhsT=x_T[1][:, nb * 128:(nb + 1) * 128],
                             rhs=w_out_sb[1][:], start=False, stop=False)
            nc.tensor.matmul(op_[:], lhsT=ones_sb[0:1, 0:128], rhs=b_out_sb[0:1, :],
                             start=False, stop=True)
            ob = work.tile([128, OUT_NF], f32, tag="ob", name=f"ob_{nb}")
            nc.vector.tensor_copy(ob[:], op_[:])
            dma(out=out_d[nb * 128:(nb + 1) * 128, :], in_=ob[:])

    if split_waits:
        _split_excess_waits(nc, mybir)
    return nc


def _get_nc():
    if "nc" not in _CACHE:
        _CACHE["nc"] = _build_nc()
    return _CACHE["nc"]


def _to_mdt(a):
    if MM_DT == "bf16":
        import ml_dtypes
        return np.asarray(a, dtype=np.float32).astype(ml_dtypes.bfloat16)
    return np.asarray(a, dtype=np.float32)


def _run_on_hw(inputs, **spmd_kwargs):
    """Shard, run on the 8 NeuronCores, gather. Returns (out, BassKernelResults)."""
    from concourse.bass_utils import run_bass_kernel_spmd

    f = np.float32
    h = np.ascontiguousarray(np.asarray(inputs["h"], dtype=f))
    ws = {k: np.ascontiguousarray(np.asarray(inputs[k], dtype=f))
          for k in ("w_in", "b_in", "w_out", "b_out", "we1", "be1", "we2",
                    "be2", "wn1", "bn1", "wn2", "bn2")}
    nc = _get_nc()
    base = {
        "w_in": _to_mdt(ws["w_in"]), "b_in": ws["b_in"],
        "w_out": ws["w_out"], "b_out": ws["b_out"],
        "we1": _to_mdt(ws["we1"]), "be1": ws["be1"],
        "we2": _to_mdt(ws["we2"]), "be2": ws["be2"],
        "wn1": _to_mdt(ws["wn1"]), "bn1": ws["bn1"],
        "wn2": _to_mdt(ws["wn2"]), "bn2": ws["bn2"],
        "ident": _to_mdt(np.eye(128, dtype=f)),
        "identf": np.eye(128, dtype=f),
        "ones_r": np.ones((1, 128), dtype=f),
    }
    in_maps = []
    for c in range(NCORES):
        m = dict(base)
        m["h_c"] = np.ascontiguousarray(
            h[c * GPC:(c + 1) * GPC].reshape(NODES, IN_NF))
        in_maps.append(m)

    res = run_bass_kernel_spmd(nc, in_maps, list(range(NCORES)), **spmd_kwargs)
    out = np.concatenate([np.asarray(res.results[i]["out_c"], dtype=f)
                          for i in range(NCORES)], axis=0)
    return out, res


def kernel(**inputs):
    h = np.asarray(inputs["h"])
    rows, cols = inputs["rows"], inputs["cols"]
    if h.shape != (BS, N, IN_NF) or not _edges_match(rows, cols):
        ws = {k: np.asarray(inputs[k], dtype=np.float32)
              for k in ("w_in", "b_in", "w_out", "b_out", "we1", "be1", "we2",
                        "be2", "wn1", "bn1", "wn2", "bn2")}
        return _numpy_reference(np.asarray(h, np.float32), rows, cols, **ws)
    out, _ = _run_on_hw(inputs)
    return out



# revision 11
# speedup vs baseline: 1.0090x; 1.0090x over previous
"""Fully-connected GNN message-passing kernel for Trainium2 (8 NeuronCores).

Strategy
--------
The reference graph is fully connected (each graph: all ordered pairs i != j).
This lets us replace gather/segment_sum with dense per-graph math:

  edge-MLP layer 1:  concat([x[i], x[j]]) @ we1 == a_i + b_j
      with a = x @ we1[:H], b = x @ we1[H:]          (tiny matmuls)
  messages for ALL i,j pairs (incl. diagonal) are computed densely;
  agg_i = sum_j silu(silu(a_i+b_j+be1) @ we2 + be2) - diagonal_term_i

Sharding: data-parallel over graphs, 2 graphs per core, weights replicated.
All feature-major ("transposed") layouts on-chip: features on partitions,
nodes/edges along the free dimension.

Engine assignment (per core):
  PE   - all matmuls (edge MLP2, a/b, node MLP, embeddings)
  ACT  - the two per-edge SiLU stages (the roofline: 1 elem/lane/cycle)
  DVE  - broadcast-build of a_i+b_j, per-receiver reductions, residuals
  DMA  - input/weight loads, output store
"""

import numpy as np

# problem shapes (hardcoded per contract)
BS, N, IN_NF, H, EH, OUT_NF, L = 16, 128, 64, 256, 128, 64, 4
NCORES = 8
GPC = BS // NCORES            # graphs per core
NODES = GPC * N               # nodes per core
HC = H // 128                 # H partition chunks

# tuning knobs
SLAB_RECV = 8                 # receivers per edge slab
SLAB = SLAB_RECV * N          # slab columns (edges)
NSLABS = N // SLAB_RECV       # slabs per graph
MMQ = 512                     # matmul moving-dim slice (fp32 limit)

Z1_MODE = "pe"
MM_DT = "bf16"

_CACHE = {}


def _silu_np(x):
    return x / (1.0 + np.exp(-x))


def _canonical_edges():
    r = np.repeat(np.arange(N), N)
    c = np.tile(np.arange(N), N)
    m = r != c
    r, c = r[m], c[m]
    off = (np.arange(BS) * N)[:, None]
    rows = (r[None, :] + off).reshape(-1)
    cols = (c[None, :] + off).reshape(-1)
    return rows, cols


def _edges_match(rows, cols):
    """True if (rows, cols) describe the canonical fully-connected batch
    (any edge ordering that keeps each graph's edge block内 self-contained
    is fine for segment_sum, but we require the exact multiset of edges)."""
    er, ec = _canonical_edges()
    rows = np.asarray(rows).astype(np.int64).ravel()
    cols = np.asarray(cols).astype(np.int64).ravel()
    if rows.shape != er.shape or cols.shape != ec.shape:
        return False
    if np.array_equal(rows, er) and np.array_equal(cols, ec):
        return True
    # permuted edge list: compare sorted edge keys
    k1 = np.sort(rows * (BS * N) + cols)
    k2 = np.sort(er * (BS * N) + ec)
    return np.array_equal(k1, k2)


def _numpy_reference(h, rows, cols, w_in, b_in, w_out, b_out,
                     we1, be1, we2, be2, wn1, bn1, wn2, bn2):
    """Exact fallback (only used if inputs are not the canonical FC batch)."""
    f = np.float32
    x = h.reshape(BS * N, -1).astype(f) @ w_in.astype(f) + b_in.astype(f)
    rows = np.asarray(rows).astype(np.int64)
    cols = np.asarray(cols).astype(np.int64)
    for l in range(L):
        m = np.concatenate([x[rows], x[cols]], axis=-1)
        m = _silu_np(m @ we1[l].astype(f) + be1[l].astype(f))
        m = _silu_np(m @ we2[l].astype(f) + be2[l].astype(f))
        agg = np.zeros((BS * N, m.shape[-1]), f)
        np.add.at(agg, rows, m)
        u = np.concatenate([x, agg], axis=-1)
        u = _silu_np(u @ wn1[l].astype(f) + bn1[l].astype(f))
        u = u @ wn2[l].astype(f) + bn2[l].astype(f)
        x = x + u
    return x @ w_out.astype(f) + b_out.astype(f)


def _split_excess_waits(nc, mybir, cap=1):
    """The walrus build in this environment accepts only one sync-wait per
    instruction; move extra waits onto preceding same-engine NOPs."""
    n_split = 0
    for fn in nc.m.functions:
        for blk in fn.blocks:
            il = blk.instructions
            new = []
            changed = False
            for ins in il:
                si = ins.sync_info
                if si is not None and si.on_wait and len(si.on_wait) > cap:
                    waits = list(si.on_wait)
                    keep, extra = waits[-cap:], waits[:-cap]
                    for w in extra:
                        nop = mybir.InstNoOp(name=f"I-wsplit-{nc.next_id()}",
                                             ins=[], outs=[])
                        nop.engine = ins.engine
                        nop.sync_info = mybir.SyncInfo(on_wait=[w], on_update=[])
                        new.append(nop)
                        n_split += 1
                    ins.sync_info = mybir.SyncInfo(on_wait=keep,
                                                   on_update=list(si.on_update))
                    changed = True
                new.append(ins)
            if changed:
                il[:] = new
    return n_split


def _build_nc(split_waits=True):
    import concourse.bass as bass
    import concourse.tile as tile
    import concourse.mybir as mybir
    from contextlib import ExitStack

    f32 = mybir.dt.float32
    bf16 = mybir.dt.bfloat16
    mdt = f32 if MM_DT == "f32" else bf16
    AF = mybir.ActivationFunctionType
    ALU = mybir.AluOpType

    nc = bass.Bass()

    # ---- DRAM parameters (per core) ----
    h_d = nc.declare_dram_parameter("h_c", [NODES, IN_NF], f32, isOutput=False)
    w_in_d = nc.declare_dram_parameter("w_in", [IN_NF, H], mdt, isOutput=False)
    b_in_d = nc.declare_dram_parameter("b_in", [H], f32, isOutput=False)
    w_out_d = nc.declare_dram_parameter("w_out", [H, OUT_NF], f32, isOutput=False)
    b_out_d = nc.declare_dram_parameter("b_out", [OUT_NF], f32, isOutput=False)
    we1_d = nc.declare_dram_parameter("we1", [L, 2 * H, EH], mdt, isOutput=False)
    be1_d = nc.declare_dram_parameter("be1", [L, EH], f32, isOutput=False)
    we2_d = nc.declare_dram_parameter("we2", [L, EH, EH], mdt, isOutput=False)
    be2_d = nc.declare_dram_parameter("be2", [L, EH], f32, isOutput=False)
    wn1_d = nc.declare_dram_parameter("wn1", [L, H + EH, H], mdt, isOutput=False)
    bn1_d = nc.declare_dram_parameter("bn1", [L, H], f32, isOutput=False)
    wn2_d = nc.declare_dram_parameter("wn2", [L, H, H], mdt, isOutput=False)
    bn2_d = nc.declare_dram_parameter("bn2", [L, H], f32, isOutput=False)
    ident_d = nc.declare_dram_parameter("ident", [128, 128], mdt, isOutput=False)
    identf_d = nc.declare_dram_parameter("identf", [128, 128], f32, isOutput=False)
    ones_d = nc.declare_dram_parameter("ones_r", [1, 128], f32, isOutput=False)
    out_d = nc.declare_dram_parameter("out_c", [NODES, OUT_NF], f32, isOutput=True)

    with tile.TileContext(nc) as tc, ExitStack() as ctx:
        consts = ctx.enter_context(tc.tile_pool(name="consts", bufs=1))
        work = ctx.enter_context(tc.tile_pool(name="work", bufs=2))
        mpool = ctx.enter_context(tc.tile_pool(name="mp", bufs=2))
        xpool = ctx.enter_context(tc.tile_pool(name="xp", bufs=2))
        psum = ctx.enter_context(tc.tile_pool(name="ps", bufs=2, space="PSUM"))

        dma = nc.sync.dma_start

        # warm the ACT Silu table immediately (zero-dependency dummy op)
        warm = work.tile([1, 2], f32, tag="warm", name="warm")
        nc.vector.memset(warm[0:1, 0:1], 0.0)
        nc.scalar.activation(warm[0:1, 1:2], warm[0:1, 0:1], AF.Silu)

        # ---- input loads (h first: it heads the critical path) ----
        hns = []
        for nb in range(NODES // 128):
            hn = work.tile([128, IN_NF], f32, tag="hn", name=f"hn_{nb}")
            dma(out=hn[:], in_=h_d[nb * 128:(nb + 1) * 128, :])
            hns.append(hn)

        # ---- constant loads ----
        identf_sb = consts.tile([128, 128], f32, tag="identf", name="identf_sb")
        dma(out=identf_sb[:], in_=identf_d[:])
        w_in_sb = consts.tile([IN_NF, H], mdt, tag="w_in", name="w_in_sb")
        dma(out=w_in_sb[:], in_=w_in_d[:])
        b_in_sb = consts.tile([128, HC], f32, tag="b_in", name="b_in_sb")
        dma(out=b_in_sb[:], in_=b_in_d.rearrange("(m p) -> p m", p=128))

        ident_sb = None
        if Z1_MODE == "pe":
            if MM_DT == "f32":
                ident_sb = identf_sb
            else:
                ident_sb = consts.tile([128, 128], mdt, tag="ident", name="ident_sb")
                dma(out=ident_sb[:], in_=ident_d[:])

        # ---- input embedding: x_T[m] = (h @ w_in + b_in)^T ----
        hT = work.tile([IN_NF, NODES], mdt, tag="hT", name="hT")
        for nb in range(NODES // 128):
            hTp = psum.tile([IN_NF, 128], f32, tag=f"zp{nb}", bufs=1, name=f"hTp_{nb}")
            nc.tensor.transpose(hTp[:], hns[nb][:], identf_sb[:])
            nc.vector.tensor_copy(hT[:, nb * 128:(nb + 1) * 128], hTp[:])

        x_T = [xpool.tile([128, NODES], f32, tag=f"x{m}", name=f"x0_{m}")
               for m in range(HC)]
        xb = [xpool.tile([128, NODES], mdt, tag=f"xb{m}", name=f"xb0_{m}")
              for m in range(HC)]
        for g in range(GPC):
            gb = slice(g * N, (g + 1) * N)
            for m in range(HC):
                xp_ = psum.tile([128, N], f32, tag=f"mp{m}", bufs=1,
                                name=f"xemb_{g}_{m}")
                nc.tensor.matmul(xp_[:], lhsT=w_in_sb[:, m * 128:(m + 1) * 128],
                                 rhs=hT[:, gb], start=True, stop=True)
                nc.vector.tensor_scalar_add(x_T[m][:, gb], xp_[:], b_in_sb[:, m:m + 1])
                nc.vector.tensor_scalar_add(xb[m][:, gb], xp_[:], b_in_sb[:, m:m + 1])

        # ---- layers ----
        we1_sb, we2_sb, wn1_sb, wn2_sb = [], [], [], []
        be1_sb = consts.tile([EH, L], f32, tag="be1", name="be1_sb")
        dma(out=be1_sb[:], in_=be1_d.rearrange("l p -> p l"))
        be2_sb = consts.tile([EH, L], f32, tag="be2", name="be2_sb")
        dma(out=be2_sb[:], in_=be2_d.rearrange("l p -> p l"))
        bn1_sb = consts.tile([128, L * HC], f32, tag="bn1", name="bn1_sb")
        dma(out=bn1_sb[:], in_=bn1_d.rearrange("l (m p) -> p (l m)", p=128))
        bn2_sb = consts.tile([128, L * HC], f32, tag="bn2", name="bn2_sb")
        dma(out=bn2_sb[:], in_=bn2_d.rearrange("l (m p) -> p (l m)", p=128))
        for l in range(L):
            t1 = []
            for j in range(4):
                t = consts.tile([128, EH], mdt, tag=f"we1_{l}_{j}", name=f"we1_{l}_{j}")
                dma(out=t[:], in_=we1_d[l, j * 128:(j + 1) * 128, :])
                t1.append(t)
            we1_sb.append(t1)
            t = consts.tile([EH, EH], mdt, tag=f"we2_{l}", name=f"we2_{l}")
            dma(out=t[:], in_=we2_d[l])
            we2_sb.append(t)
            tn = []
            for k in range(3):
                t = consts.tile([128, H], mdt, tag=f"wn1_{l}_{k}", name=f"wn1_{l}_{k}")
                dma(out=t[:], in_=wn1_d[l, k * 128:(k + 1) * 128, :])
                tn.append(t)
            wn1_sb.append(tn)
            tn = []
            for k in range(2):
                t = consts.tile([128, H], mdt, tag=f"wn2_{l}_{k}", name=f"wn2_{l}_{k}")
                dma(out=t[:], in_=wn2_d[l, k * 128:(k + 1) * 128, :])
                tn.append(t)
            wn2_sb.append(tn)
        w_out_sb = []
        for k in range(HC):
            t = consts.tile([128, OUT_NF], f32, tag=f"w_out_{k}", name=f"w_out_{k}")
            dma(out=t[:], in_=w_out_d[k * 128:(k + 1) * 128, :])
            w_out_sb.append(t)
        b_out_sb = consts.tile([1, OUT_NF], f32, tag="b_out", name="b_out_sb")
        dma(out=b_out_sb[:], in_=b_out_d[:].unsqueeze(0))
        ones_sb = consts.tile([1, 128], f32, tag="ones", name="ones_sb")
        dma(out=ones_sb[:], in_=ones_d[:])

        RPQ = MMQ // N                # receivers per matmul slice
        SKEW = 4      # g0 runs ahead; g1's tail covers g0's boundary chain
        st = {("x", 0): (x_T, xb)}

        def prep(l, g):
            """a/b projections + b4 for (layer l, graph g)."""
            if ("abTm", l) not in st:
                st[("abTm", l)] = work.tile([EH, NODES], mdt, tag="abTm",
                                            name=f"abTm_{l}")
            abTm = st[("abTm", l)]
            xb_cur = st[("x", l)][1]
            gb = slice(g * N, (g + 1) * N)
            ap_ = psum.tile([EH, N], f32, tag=f"zp{g}", bufs=1,
                            name=f"ap_{l}_{g}")
            for k in range(HC):
                nc.tensor.matmul(ap_[:], lhsT=we1_sb[l][k][:],
                                 rhs=xb_cur[k][:, gb],
                                 start=(k == 0), stop=(k == HC - 1))
            nc.vector.tensor_scalar_add(abTm[:, gb], ap_[:], be1_sb[:, l:l + 1])
            bp_ = psum.tile([EH, N], f32, tag=f"mp{g}", bufs=1,
                            name=f"bp_{l}_{g}")
            for k in range(HC):
                nc.tensor.matmul(bp_[:], lhsT=we1_sb[l][HC + k][:],
                                 rhs=xb_cur[k][:, gb],
                                 start=(k == 0), stop=(k == HC - 1))
            # materialize [b, b, .., b] (RPQ copies) straight from psum
            # so the B-term of the z1 broadcast runs as one N=MMQ matmul
            b4 = work.tile([EH, MMQ], mdt, tag=f"b4_{g}", name=f"b4_{l}_{g}")
            nc.scalar.activation(
                b4[:].rearrange("p (r j) -> p r j", r=RPQ),
                bp_[:].unsqueeze(1).broadcast_to([EH, RPQ, N]), AF.Copy)
            st[("b4", l, g)] = b4

        def slab(l, s, g):
            """z1 -> silu1 -> mm2 -> silu2 -> reduce/diag for one slab."""
            if ("agg", l) not in st:
                st[("agg", l)] = work.tile([EH, NODES], f32, tag="agg",
                                           name=f"agg_{l}")
                st[("diag", l)] = work.tile([EH, NODES], f32, tag="diag",
                                            name=f"diag_{l}")
            abTm, b4 = st[("abTm", l)], st[("b4", l, g)]
            aggT, diagT = st[("agg", l)], st[("diag", l)]
            r0 = s * SLAB_RECV
            zp = psum.tile([EH, SLAB], f32, tag=f"zp{g}", bufs=1,
                           name=f"zp_{l}_{g}_{s}")
            for q in range(SLAB // MMQ):
                base = g * N + r0 + RPQ * q
                rhs_a = abTm[:, base: base + RPQ] \
                    .unsqueeze(2).broadcast_to([EH, RPQ, N])
                nc.tensor.matmul(zp[:, q * MMQ:(q + 1) * MMQ],
                                 lhsT=ident_sb[:], rhs=rhs_a,
                                 start=True, stop=False)
                nc.tensor.matmul(zp[:, q * MMQ:(q + 1) * MMQ],
                                 lhsT=ident_sb[:], rhs=b4[:],
                                 start=False, stop=True)
            m1 = mpool.tile([EH, SLAB], mdt, tag=f"m1_{g}", bufs=3,
                            name=f"m1_{l}_{g}_{s}")
            nc.scalar.activation(m1[:], zp[:], AF.Silu)
            mp_ = psum.tile([EH, SLAB], f32, tag=f"mp{g}", bufs=1,
                            name=f"mp_{l}_{g}_{s}")
            for q in range(SLAB // MMQ):
                nc.tensor.matmul(mp_[:, q * MMQ:(q + 1) * MMQ],
                                 lhsT=we2_sb[l][:],
                                 rhs=m1[:, q * MMQ:(q + 1) * MMQ],
                                 start=True, stop=True)
            m2 = mpool.tile([EH, SLAB], f32, tag=f"m2_{g}", bufs=3,
                            name=f"m2_{l}_{g}_{s}")
            nc.scalar.activation(m2[:], mp_[:], AF.Silu, bias=be2_sb[:, l:l + 1])
            red_in = m2[:].rearrange("p (i j) -> p i j", j=N)
            nc.vector.tensor_reduce(
                aggT[:, g * N + r0: g * N + r0 + SLAB_RECV],
                red_in, axis=mybir.AxisListType.X, op=ALU.add)
            # pull out the diagonal (j == i) messages for correction:
            # within this slab, receiver r0+r's own column is r*N+r0+r
            diag_ap = bass.AP(
                tensor=m2.tensor, offset=m2.offset + r0,
                ap=[m2.ap[0], [N + 1, SLAB_RECV]])
            nc.vector.tensor_copy(
                diagT[:, g * N + r0: g * N + r0 + SLAB_RECV], diag_ap)

        def node(l, g):
            """diag correction + node MLP + residual for (layer l, graph g)."""
            if ("u1", l) not in st:
                st[("u1", l)] = [work.tile([128, NODES], mdt, tag=f"u1_{m}",
                                           name=f"u1_{l}_{m}")
                                 for m in range(HC)]
                st[("x", l + 1)] = (
                    [xpool.tile([128, NODES], f32, tag=f"x{m}",
                                name=f"x{l + 1}_{m}") for m in range(HC)],
                    [xpool.tile([128, NODES], mdt, tag=f"xb{m}",
                                name=f"xb{l + 1}_{m}") for m in range(HC)])
            u1 = st[("u1", l)]
            aggT, diagT = st[("agg", l)], st[("diag", l)]
            x_cur, xb_cur = st[("x", l)]
            x_new, xb_new = st[("x", l + 1)]
            gb = slice(g * N, (g + 1) * N)
            nc.vector.tensor_sub(aggT[:, gb], aggT[:, gb], diagT[:, gb])
            aggb = work.tile([EH, N], mdt, tag=f"aggb{g}", name=f"aggb_{l}_{g}")
            nc.vector.tensor_copy(aggb[:], aggT[:, gb])
            for m in range(HC):
                tag = f"zp{g}" if m == 0 else f"mp{g}"
                up = psum.tile([128, N], f32, tag=tag, bufs=1,
                               name=f"u1p_{l}_{g}_{m}")
                nc.tensor.matmul(up[:], lhsT=wn1_sb[l][0][:, m * 128:(m + 1) * 128],
                                 rhs=xb_cur[0][:, gb], start=True, stop=False)
                nc.tensor.matmul(up[:], lhsT=wn1_sb[l][1][:, m * 128:(m + 1) * 128],
                                 rhs=xb_cur[1][:, gb], start=False, stop=False)
                nc.tensor.matmul(up[:], lhsT=wn1_sb[l][2][:, m * 128:(m + 1) * 128],
                                 rhs=aggb[:], start=False, stop=True)
                nc.scalar.activation(u1[m][:, gb], up[:], AF.Silu,
                                     bias=bn1_sb[:, l * HC + m: l * HC + m + 1])
            for m in range(HC):
                tag = f"zp{g}" if m == 0 else f"mp{g}"
                u2p = psum.tile([128, N], f32, tag=tag, bufs=1,
                                name=f"u2p_{l}_{g}_{m}")
                nc.tensor.matmul(u2p[:], lhsT=wn2_sb[l][0][:, m * 128:(m + 1) * 128],
                                 rhs=u1[0][:, gb], start=True, stop=False)
                nc.tensor.matmul(u2p[:], lhsT=wn2_sb[l][1][:, m * 128:(m + 1) * 128],
                                 rhs=u1[1][:, gb], start=False, stop=True)
                nc.vector.scalar_tensor_tensor(
                    x_new[m][:, gb], u2p[:],
                    bn2_sb[:, l * HC + m: l * HC + m + 1], x_cur[m][:, gb],
                    op0=ALU.add, op1=ALU.add)
                nc.vector.tensor_copy(xb_new[m][:, gb], x_new[m][:, gb])

        def out(g):
            x_fin = st[("x", L)][0]
            op_ = psum.tile([128, OUT_NF], f32, tag=f"zp{g}", bufs=1,
                            name=f"outp_{g}")
            nc.tensor.matmul(op_[:], lhsT=x_fin[0][:, g * 128:(g + 1) * 128],
                             rhs=w_out_sb[0][:], start=True, stop=False)
            nc.tensor.matmul(op_[:], lhsT=x_fin[1][:, g * 128:(g + 1) * 128],
                             rhs=w_out_sb[1][:], start=False, stop=False)
            nc.tensor.matmul(op_[:], lhsT=ones_sb[0:1, 0:128], rhs=b_out_sb[0:1, :],
                             start=False, stop=True)
            ob = work.tile([128, OUT_NF], f32, tag="ob", name=f"ob_{g}")
            nc.vector.tensor_copy(ob[:], op_[:])
            dma(out=out_d[g * 128:(g + 1) * 128, :], in_=ob[:])

        # Emission schedule: the two graphs run as staggered pipelines.
        # g0's node-MLP and next-layer a/b prep ride inside g1's slab tail;
        # g1's boundary work rides inside the next layer's g0 warmup slabs.
        events = [("prep", 0, 0), ("prep", 0, 1)]
        for l in range(L):
            events += [("slab", l, 0, 0), ("slab", l, 1, 0)]
            if l > 0:
                events += [("node", l - 1, 1), ("prep", l, 1)]
            events += [("slab", l, 2, 0), ("slab", l, 3, 0)]
            for s in range(NSLABS):
                events.append(("slab", l, s, 1))
                if s + SKEW < NSLABS:
                    events.append(("slab", l, s + SKEW, 0))
                if s == NSLABS - SKEW:
                    events.append(("node", l, 0))
                if s == NSLABS - SKEW + 1:
                    events.append(("prep", l + 1, 0) if l + 1 < L
                                  else ("out", 0))
        events += [("node", L - 1, 1), ("out", 1)]

        for ev in events:
            if ev[0] == "prep":
                prep(ev[1], ev[2])
            elif ev[0] == "slab":
                slab(ev[1], ev[2], ev[3])
            elif ev[0] == "node":
                node(ev[1], ev[2])
            else:
                out(ev[1])

    if split_waits:
        _split_excess_waits(nc, mybir)
    return nc


def _get_nc():
    if "nc" not in _CACHE:
        _CACHE["nc"] = _build_nc()
    return _CACHE["nc"]


def _to_mdt(a):
    if MM_DT == "bf16":
        import ml_dtypes
        return np.asarray(a, dtype=np.float32).astype(ml_dtypes.bfloat16)
    return np.asarray(a, dtype=np.float32)


def _run_on_hw(inputs, **spmd_kwargs):
    """Shard, run on the 8 NeuronCores, gather. Returns (out, BassKernelResults)."""
    from concourse.bass_utils import run_bass_kernel_spmd

    f = np.float32
    h = np.ascontiguousarray(np.asarray(inputs["h"], dtype=f))
    ws = {k: np.ascontiguousarray(np.asarray(inputs[k], dtype=f))
          for k in ("w_in", "b_in", "w_out", "b_out", "we1", "be1", "we2",
                    "be2", "wn1", "bn1", "wn2", "bn2")}
    nc = _get_nc()
    base = {
        "w_in": _to_mdt(ws["w_in"]), "b_in": ws["b_in"],
        "w_out": ws["w_out"], "b_out": ws["b_out"],
        "we1": _to_mdt(ws["we1"]), "be1": ws["be1"],
        "we2": _to_mdt(ws["we2"]), "be2": ws["be2"],
        "wn1": _to_mdt(ws["wn1"]), "bn1": ws["bn1"],
        "wn2": _to_mdt(ws["wn2"]), "bn2": ws["bn2"],
        "ident": _to_mdt(np.eye(128, dtype=f)),
        "identf": np.eye(128, dtype=f),
        "ones_r": np.ones((1, 128), dtype=f),
    }
    in_maps = []
    for c in range(NCORES):
        m = dict(base)
        m["h_c"] = np.ascontiguousarray(
            h[c * GPC:(c + 1) * GPC].reshape(NODES, IN_NF))
        in_maps.append(m)

    res = run_bass_kernel_spmd(nc, in_maps, list(range(NCORES)), **spmd_kwargs)
    out = np.concatenate([np.asarray(res.results[i]["out_c"], dtype=f)
                          for i in range(NCORES)], axis=0)
    return out, res


def kernel(**inputs):
    h = np.asarray(inputs["h"])
    rows, cols = inputs["rows"], inputs["cols"]
    if h.shape != (BS, N, IN_NF) or not _edges_match(rows, cols):
        ws = {k: np.asarray(inputs[k], dtype=np.float32)
              for k in ("w_in", "b_in", "w_out", "b_out", "we1", "be1", "we2",
                        "be2", "wn1", "bn1", "wn2", "bn2")}
        return _numpy_reference(np.asarray(h, np.float32), rows, cols, **ws)
    out, _ = _run_on_hw(inputs)
    return out



# revision 13
# speedup vs baseline: 1.0093x; 1.0003x over previous
"""Fully-connected GNN message-passing kernel for Trainium2 (8 NeuronCores).

Strategy
--------
The reference graph is fully connected (each graph: all ordered pairs i != j).
This lets us replace gather/segment_sum with dense per-graph math:

  edge-MLP layer 1:  concat([x[i], x[j]]) @ we1 == a_i + b_j
      with a = x @ we1[:H], b = x @ we1[H:]          (tiny matmuls)
  messages for ALL i,j pairs (incl. diagonal) are computed densely;
  agg_i = sum_j silu(silu(a_i+b_j+be1) @ we2 + be2) - diagonal_term_i

Sharding: data-parallel over graphs, 2 graphs per core, weights replicated.
All feature-major ("transposed") layouts on-chip: features on partitions,
nodes/edges along the free dimension.

Engine assignment (per core):
  PE   - all matmuls (edge MLP2, a/b, node MLP, embeddings)
  ACT  - the two per-edge SiLU stages (the roofline: 1 elem/lane/cycle)
  DVE  - broadcast-build of a_i+b_j, per-receiver reductions, residuals
  DMA  - input/weight loads, output store
"""

import numpy as np

# problem shapes (hardcoded per contract)
BS, N, IN_NF, H, EH, OUT_NF, L = 16, 128, 64, 256, 128, 64, 4
NCORES = 8
GPC = BS // NCORES            # graphs per core
NODES = GPC * N               # nodes per core
HC = H // 128                 # H partition chunks

# tuning knobs
SLAB_RECV = 8                 # receivers per edge slab
SLAB = SLAB_RECV * N          # slab columns (edges)
NSLABS = N // SLAB_RECV       # slabs per graph
MMQ = 512                     # matmul moving-dim slice (fp32 limit)

Z1_MODE = "pe"
MM_DT = "bf16"

_CACHE = {}


def _silu_np(x):
    return x / (1.0 + np.exp(-x))


def _canonical_edges():
    r = np.repeat(np.arange(N), N)
    c = np.tile(np.arange(N), N)
    m = r != c
    r, c = r[m], c[m]
    off = (np.arange(BS) * N)[:, None]
    rows = (r[None, :] + off).reshape(-1)
    cols = (c[None, :] + off).reshape(-1)
    return rows, cols


def _edges_match(rows, cols):
    """True if (rows, cols) describe the canonical fully-connected batch
    (any edge ordering that keeps each graph's edge block内 self-contained
    is fine for segment_sum, but we require the exact multiset of edges)."""
    er, ec = _canonical_edges()
    rows = np.asarray(rows).astype(np.int64).ravel()
    cols = np.asarray(cols).astype(np.int64).ravel()
    if rows.shape != er.shape or cols.shape != ec.shape:
        return False
    if np.array_equal(rows, er) and np.array_equal(cols, ec):
        return True
    # permuted edge list: compare sorted edge keys
    k1 = np.sort(rows * (BS * N) + cols)
    k2 = np.sort(er * (BS * N) + ec)
    return np.array_equal(k1, k2)


def _numpy_reference(h, rows, cols, w_in, b_in, w_out, b_out,
                     we1, be1, we2, be2, wn1, bn1, wn2, bn2):
    """Exact fallback (only used if inputs are not the canonical FC batch)."""
    f = np.float32
    x = h.reshape(BS * N, -1).astype(f) @ w_in.astype(f) + b_in.astype(f)
    rows = np.asarray(rows).astype(np.int64)
    cols = np.asarray(cols).astype(np.int64)
    for l in range(L):
        m = np.concatenate([x[rows], x[cols]], axis=-1)
        m = _silu_np(m @ we1[l].astype(f) + be1[l].astype(f))
        m = _silu_np(m @ we2[l].astype(f) + be2[l].astype(f))
        agg = np.zeros((BS * N, m.shape[-1]), f)
        np.add.at(agg, rows, m)
        u = np.concatenate([x, agg], axis=-1)
        u = _silu_np(u @ wn1[l].astype(f) + bn1[l].astype(f))
        u = u @ wn2[l].astype(f) + bn2[l].astype(f)
        x = x + u
    return x @ w_out.astype(f) + b_out.astype(f)


def _split_excess_waits(nc, mybir, cap=1):
    """The walrus build in this environment accepts only one sync-wait per
    instruction; move extra waits onto preceding same-engine NOPs."""
    n_split = 0
    for fn in nc.m.functions:
        for blk in fn.blocks:
            il = blk.instructions
            new = []
            changed = False
            for ins in il:
                si = ins.sync_info
                if si is not None and si.on_wait and len(si.on_wait) > cap:
                    waits = list(si.on_wait)
                    keep, extra = waits[-cap:], waits[:-cap]
                    for w in extra:
                        nop = mybir.InstNoOp(name=f"I-wsplit-{nc.next_id()}",
                                             ins=[], outs=[])
                        nop.engine = ins.engine
                        nop.sync_info = mybir.SyncInfo(on_wait=[w], on_update=[])
                        new.append(nop)
                        n_split += 1
                    ins.sync_info = mybir.SyncInfo(on_wait=keep,
                                                   on_update=list(si.on_update))
                    changed = True
                new.append(ins)
            if changed:
                il[:] = new
    return n_split


def _build_nc(split_waits=True):
    import concourse.bass as bass
    import concourse.tile as tile
    import concourse.mybir as mybir
    from contextlib import ExitStack

    f32 = mybir.dt.float32
    bf16 = mybir.dt.bfloat16
    mdt = f32 if MM_DT == "f32" else bf16
    AF = mybir.ActivationFunctionType
    ALU = mybir.AluOpType

    nc = bass.Bass()

    # ---- DRAM parameters (per core) ----
    h_d = nc.declare_dram_parameter("h_c", [NODES, IN_NF], f32, isOutput=False)
    w_in_d = nc.declare_dram_parameter("w_in", [IN_NF, H], mdt, isOutput=False)
    b_in_d = nc.declare_dram_parameter("b_in", [H], f32, isOutput=False)
    w_out_d = nc.declare_dram_parameter("w_out", [H, OUT_NF], f32, isOutput=False)
    b_out_d = nc.declare_dram_parameter("b_out", [OUT_NF], f32, isOutput=False)
    we1_d = nc.declare_dram_parameter("we1", [L, 2 * H, EH], mdt, isOutput=False)
    be1_d = nc.declare_dram_parameter("be1", [L, EH], f32, isOutput=False)
    we2_d = nc.declare_dram_parameter("we2", [L, EH, EH], mdt, isOutput=False)
    be2_d = nc.declare_dram_parameter("be2", [L, EH], f32, isOutput=False)
    wn1_d = nc.declare_dram_parameter("wn1", [L, H + EH, H], mdt, isOutput=False)
    bn1_d = nc.declare_dram_parameter("bn1", [L, H], f32, isOutput=False)
    wn2_d = nc.declare_dram_parameter("wn2", [L, H, H], mdt, isOutput=False)
    bn2_d = nc.declare_dram_parameter("bn2", [L, H], f32, isOutput=False)
    ident_d = nc.declare_dram_parameter("ident", [128, 128], mdt, isOutput=False)
    identf_d = nc.declare_dram_parameter("identf", [128, 128], f32, isOutput=False)
    ones_d = nc.declare_dram_parameter("ones_r", [1, 128], f32, isOutput=False)
    out_d = nc.declare_dram_parameter("out_c", [NODES, OUT_NF], f32, isOutput=True)

    with tile.TileContext(nc) as tc, ExitStack() as ctx:
        consts = ctx.enter_context(tc.tile_pool(name="consts", bufs=1))
        work = ctx.enter_context(tc.tile_pool(name="work", bufs=2))
        mpool = ctx.enter_context(tc.tile_pool(name="mp", bufs=2))
        xpool = ctx.enter_context(tc.tile_pool(name="xp", bufs=2))
        psum = ctx.enter_context(tc.tile_pool(name="ps", bufs=2, space="PSUM"))

        dma = nc.sync.dma_start

        # warm the ACT Silu table immediately (zero-dependency dummy op)
        warm = work.tile([1, 2], f32, tag="warm", name="warm")
        nc.vector.memset(warm[0:1, 0:1], 0.0)
        nc.scalar.activation(warm[0:1, 1:2], warm[0:1, 0:1], AF.Silu)

        # ---- input loads (h first: it heads the critical path) ----
        hns = []
        for nb in range(NODES // 128):
            hn = work.tile([128, IN_NF], f32, tag="hn", name=f"hn_{nb}")
            dma(out=hn[:], in_=h_d[nb * 128:(nb + 1) * 128, :])
            hns.append(hn)

        # ---- constant loads ----
        identf_sb = consts.tile([128, 128], f32, tag="identf", name="identf_sb")
        dma(out=identf_sb[:], in_=identf_d[:])
        w_in_sb = consts.tile([IN_NF, H], mdt, tag="w_in", name="w_in_sb")
        dma(out=w_in_sb[:], in_=w_in_d[:])
        b_in_sb = consts.tile([128, HC], f32, tag="b_in", name="b_in_sb")
        dma(out=b_in_sb[:], in_=b_in_d.rearrange("(m p) -> p m", p=128))

        ident_sb = None
        if Z1_MODE == "pe":
            if MM_DT == "f32":
                ident_sb = identf_sb
            else:
                ident_sb = consts.tile([128, 128], mdt, tag="ident", name="ident_sb")
                dma(out=ident_sb[:], in_=ident_d[:])

        # ---- input embedding: x_T[m] = (h @ w_in + b_in)^T ----
        hT = work.tile([IN_NF, NODES], mdt, tag="hT", name="hT")
        for nb in range(NODES // 128):
            hTp = psum.tile([IN_NF, 128], f32, tag=f"zp{nb}", bufs=1, name=f"hTp_{nb}")
            nc.tensor.transpose(hTp[:], hns[nb][:], identf_sb[:])
            nc.vector.tensor_copy(hT[:, nb * 128:(nb + 1) * 128], hTp[:])

        x_T = [xpool.tile([128, NODES], f32, tag=f"x{m}", name=f"x0_{m}")
               for m in range(HC)]
        xb = [xpool.tile([128, NODES], mdt, tag=f"xb{m}", name=f"xb0_{m}")
              for m in range(HC)]
        for g in range(GPC):
            gb = slice(g * N, (g + 1) * N)
            for m in range(HC):
                xp_ = psum.tile([128, N], f32, tag=f"mp{m}", bufs=1,
                                name=f"xemb_{g}_{m}")
                nc.tensor.matmul(xp_[:], lhsT=w_in_sb[:, m * 128:(m + 1) * 128],
                                 rhs=hT[:, gb], start=True, stop=True)
                nc.vector.tensor_scalar_add(x_T[m][:, gb], xp_[:], b_in_sb[:, m:m + 1])
                nc.vector.tensor_scalar_add(xb[m][:, gb], xp_[:], b_in_sb[:, m:m + 1])

        # ---- layers ----
        we1_sb, we2_sb, wn1_sb, wn2_sb = [], [], [], []
        be1_sb = consts.tile([EH, L], f32, tag="be1", name="be1_sb")
        dma(out=be1_sb[:], in_=be1_d.rearrange("l p -> p l"))
        be2_sb = consts.tile([EH, L], f32, tag="be2", name="be2_sb")
        bn1_sb = consts.tile([128, L * HC], f32, tag="bn1", name="bn1_sb")
        bn2_sb = consts.tile([128, L * HC], f32, tag="bn2", name="bn2_sb")
        for l in range(L):
            t1 = []
            for j in range(4):
                t = consts.tile([128, EH], mdt, tag=f"we1_{l}_{j}", name=f"we1_{l}_{j}")
                dma(out=t[:], in_=we1_d[l, j * 128:(j + 1) * 128, :])
                t1.append(t)
            we1_sb.append(t1)
            t = consts.tile([EH, EH], mdt, tag=f"we2_{l}", name=f"we2_{l}")
            dma(out=t[:], in_=we2_d[l])
            we2_sb.append(t)
            tn = []
            for k in range(3):
                t = consts.tile([128, H], mdt, tag=f"wn1_{l}_{k}", name=f"wn1_{l}_{k}")
                dma(out=t[:], in_=wn1_d[l, k * 128:(k + 1) * 128, :])
                tn.append(t)
            wn1_sb.append(tn)
            tn = []
            for k in range(2):
                t = consts.tile([128, H], mdt, tag=f"wn2_{l}_{k}", name=f"wn2_{l}_{k}")
                dma(out=t[:], in_=wn2_d[l, k * 128:(k + 1) * 128, :])
                tn.append(t)
            wn2_sb.append(tn)
            if l == 0:
                dma(out=be2_sb[:], in_=be2_d.rearrange("l p -> p l"))
                dma(out=bn1_sb[:],
                    in_=bn1_d.rearrange("l (m p) -> p (l m)", p=128))
                dma(out=bn2_sb[:],
                    in_=bn2_d.rearrange("l (m p) -> p (l m)", p=128))
        w_out_sb = []
        for k in range(HC):
            t = consts.tile([128, OUT_NF], f32, tag=f"w_out_{k}", name=f"w_out_{k}")
            dma(out=t[:], in_=w_out_d[k * 128:(k + 1) * 128, :])
            w_out_sb.append(t)
        b_out_sb = consts.tile([1, OUT_NF], f32, tag="b_out", name="b_out_sb")
        dma(out=b_out_sb[:], in_=b_out_d[:].unsqueeze(0))
        ones_sb = consts.tile([1, 128], f32, tag="ones", name="ones_sb")
        dma(out=ones_sb[:], in_=ones_d[:])

        RPQ = MMQ // N                # receivers per matmul slice
        SKEW = 5      # g0 runs ahead; g1's tail covers g0's boundary chain
        st = {("x", 0): (x_T, xb)}

        def prep(l, g):
            """a/b projections + b4 for (layer l, graph g)."""
            if ("abTm", l) not in st:
                st[("abTm", l)] = work.tile([EH, NODES], mdt, tag="abTm",
                                            name=f"abTm_{l}")
            abTm = st[("abTm", l)]
            xb_cur = st[("x", l)][1]
            gb = slice(g * N, (g + 1) * N)
            ap_ = psum.tile([EH, N], f32, tag=f"zp{g}", bufs=1,
                            name=f"ap_{l}_{g}")
            for k in range(HC):
                nc.tensor.matmul(ap_[:], lhsT=we1_sb[l][k][:],
                                 rhs=xb_cur[k][:, gb],
                                 start=(k == 0), stop=(k == HC - 1))
            nc.vector.tensor_scalar_add(abTm[:, gb], ap_[:], be1_sb[:, l:l + 1])
            bp_ = psum.tile([EH, N], f32, tag=f"mp{g}", bufs=1,
                            name=f"bp_{l}_{g}")
            for k in range(HC):
                nc.tensor.matmul(bp_[:], lhsT=we1_sb[l][HC + k][:],
                                 rhs=xb_cur[k][:, gb],
                                 start=(k == 0), stop=(k == HC - 1))
            # materialize [b, b, .., b] (RPQ copies) straight from psum
            # so the B-term of the z1 broadcast runs as one N=MMQ matmul
            b4 = work.tile([EH, MMQ], mdt, tag=f"b4_{g}", name=f"b4_{l}_{g}")
            nc.scalar.activation(
                b4[:].rearrange("p (r j) -> p r j", r=RPQ),
                bp_[:].unsqueeze(1).broadcast_to([EH, RPQ, N]), AF.Copy)
            st[("b4", l, g)] = b4

        def slab(l, s, g):
            """z1 -> silu1 -> mm2 -> silu2 -> reduce/diag for one slab."""
            if ("agg", l) not in st:
                st[("agg", l)] = work.tile([EH, NODES], f32, tag="agg",
                                           name=f"agg_{l}")
                st[("diag", l)] = work.tile([EH, NODES], f32, tag="diag",
                                            name=f"diag_{l}")
            abTm, b4 = st[("abTm", l)], st[("b4", l, g)]
            aggT, diagT = st[("agg", l)], st[("diag", l)]
            r0 = s * SLAB_RECV
            zp = psum.tile([EH, SLAB], f32, tag=f"zp{g}", bufs=1,
                           name=f"zp_{l}_{g}_{s}")
            for q in range(SLAB // MMQ):
                base = g * N + r0 + RPQ * q
                rhs_a = abTm[:, base: base + RPQ] \
                    .unsqueeze(2).broadcast_to([EH, RPQ, N])
                nc.tensor.matmul(zp[:, q * MMQ:(q + 1) * MMQ],
                                 lhsT=ident_sb[:], rhs=rhs_a,
                                 start=True, stop=False)
                nc.tensor.matmul(zp[:, q * MMQ:(q + 1) * MMQ],
                                 lhsT=ident_sb[:], rhs=b4[:],
                                 start=False, stop=True)
            m1 = mpool.tile([EH, SLAB], mdt, tag=f"m1_{g}", bufs=3,
                            name=f"m1_{l}_{g}_{s}")
            nc.scalar.activation(m1[:], zp[:], AF.Silu)
            mp_ = psum.tile([EH, SLAB], f32, tag=f"mp{g}", bufs=1,
                            name=f"mp_{l}_{g}_{s}")
            for q in range(SLAB // MMQ):
                nc.tensor.matmul(mp_[:, q * MMQ:(q + 1) * MMQ],
                                 lhsT=we2_sb[l][:],
                                 rhs=m1[:, q * MMQ:(q + 1) * MMQ],
                                 start=True, stop=True)
            m2 = mpool.tile([EH, SLAB], f32, tag=f"m2_{g}", bufs=3,
                            name=f"m2_{l}_{g}_{s}")
            nc.scalar.activation(m2[:], mp_[:], AF.Silu, bias=be2_sb[:, l:l + 1])
            red_in = m2[:].rearrange("p (i j) -> p i j", j=N)
            nc.vector.tensor_reduce(
                aggT[:, g * N + r0: g * N + r0 + SLAB_RECV],
                red_in, axis=mybir.AxisListType.X, op=ALU.add)
            # pull out the diagonal (j == i) messages for correction:
            # within this slab, receiver r0+r's own column is r*N+r0+r
            diag_ap = bass.AP(
                tensor=m2.tensor, offset=m2.offset + r0,
                ap=[m2.ap[0], [N + 1, SLAB_RECV]])
            nc.vector.tensor_copy(
                diagT[:, g * N + r0: g * N + r0 + SLAB_RECV], diag_ap)

        def node(l, g):
            """diag correction + node MLP + residual for (layer l, graph g)."""
            if ("u1", l) not in st:
                st[("u1", l)] = [work.tile([128, NODES], mdt, tag=f"u1_{m}",
                                           name=f"u1_{l}_{m}")
                                 for m in range(HC)]
                st[("x", l + 1)] = (
                    [xpool.tile([128, NODES], f32, tag=f"x{m}",
                                name=f"x{l + 1}_{m}") for m in range(HC)],
                    [xpool.tile([128, NODES], mdt, tag=f"xb{m}",
                                name=f"xb{l + 1}_{m}") for m in range(HC)])
            u1 = st[("u1", l)]
            aggT, diagT = st[("agg", l)], st[("diag", l)]
            x_cur, xb_cur = st[("x", l)]
            x_new, xb_new = st[("x", l + 1)]
            gb = slice(g * N, (g + 1) * N)
            nc.vector.tensor_sub(aggT[:, gb], aggT[:, gb], diagT[:, gb])
            aggb = work.tile([EH, N], mdt, tag=f"aggb{g}", name=f"aggb_{l}_{g}")
            nc.vector.tensor_copy(aggb[:], aggT[:, gb])
            for m in range(HC):
                tag = f"zp{g}" if m == 0 else f"mp{g}"
                up = psum.tile([128, N], f32, tag=tag, bufs=1,
                               name=f"u1p_{l}_{g}_{m}")
                nc.tensor.matmul(up[:], lhsT=wn1_sb[l][0][:, m * 128:(m + 1) * 128],
                                 rhs=xb_cur[0][:, gb], start=True, stop=False)
                nc.tensor.matmul(up[:], lhsT=wn1_sb[l][1][:, m * 128:(m + 1) * 128],
                                 rhs=xb_cur[1][:, gb], start=False, stop=False)
                nc.tensor.matmul(up[:], lhsT=wn1_sb[l][2][:, m * 128:(m + 1) * 128],
                                 rhs=aggb[:], start=False, stop=True)
                nc.scalar.activation(u1[m][:, gb], up[:], AF.Silu,
                                     bias=bn1_sb[:, l * HC + m: l * HC + m + 1])
            for m in range(HC):
                tag = f"zp{g}" if m == 0 else f"mp{g}"
                u2p = psum.tile([128, N], f32, tag=tag, bufs=1,
                                name=f"u2p_{l}_{g}_{m}")
                nc.tensor.matmul(u2p[:], lhsT=wn2_sb[l][0][:, m * 128:(m + 1) * 128],
                                 rhs=u1[0][:, gb], start=True, stop=False)
                nc.tensor.matmul(u2p[:], lhsT=wn2_sb[l][1][:, m * 128:(m + 1) * 128],
                                 rhs=u1[1][:, gb], start=False, stop=True)
                nc.vector.scalar_tensor_tensor(
                    x_new[m][:, gb], u2p[:],
                    bn2_sb[:, l * HC + m: l * HC + m + 1], x_cur[m][:, gb],
                    op0=ALU.add, op1=ALU.add)
                nc.vector.tensor_copy(xb_new[m][:, gb], x_new[m][:, gb])

        def out(g):
            x_fin = st[("x", L)][0]
            op_ = psum.tile([128, OUT_NF], f32, tag=f"zp{g}", bufs=1,
                            name=f"outp_{g}")
            nc.tensor.matmul(op_[:], lhsT=x_fin[0][:, g * 128:(g + 1) * 128],
                             rhs=w_out_sb[0][:], start=True, stop=False)
            nc.tensor.matmul(op_[:], lhsT=x_fin[1][:, g * 128:(g + 1) * 128],
                             rhs=w_out_sb[1][:], start=False, stop=False)
            nc.tensor.matmul(op_[:], lhsT=ones_sb[0:1, 0:128], rhs=b_out_sb[0:1, :],
                             start=False, stop=True)
            ob = work.tile([128, OUT_NF], f32, tag="ob", name=f"ob_{g}")
            nc.vector.tensor_copy(ob[:], op_[:])
            dma(out=out_d[g * 128:(g + 1) * 128, :], in_=ob[:])

        # Emission schedule: the two graphs run as staggered pipelines.
        # g0's node-MLP and next-layer a/b prep ride inside g1's slab tail;
        # g1's boundary work rides inside the next layer's g0 warmup slabs.
        events = [("prep", 0, 0), ("prep", 0, 1)]
        for l in range(L):
            events += [("slab", l, 0, 0), ("slab", l, 1, 0)]
            if l > 0:
                events += [("node", l - 1, 1), ("prep", l, 1)]
            events += [("slab", l, s, 0) for s in range(2, SKEW)]
            for s in range(NSLABS):
                events.append(("slab", l, s, 1))
                if s + SKEW < NSLABS:
                    events.append(("slab", l, s + SKEW, 0))
                if s == NSLABS - SKEW:
                    events.append(("node", l, 0))
                if s == NSLABS - SKEW + 2:
                    events.append(("prep", l + 1, 0) if l + 1 < L
                                  else ("out", 0))
        events += [("node", L - 1, 1), ("out", 1)]

        for ev in events:
            if ev[0] == "prep":
                prep(ev[1], ev[2])
            elif ev[0] == "slab":
                slab(ev[1], ev[2], ev[3])
            elif ev[0] == "node":
                node(ev[1], ev[2])
            else:
                out(ev[1])

    if split_waits:
        _split_excess_waits(nc, mybir)
    return nc


def _get_nc():
    if "nc" not in _CACHE:
        _CACHE["nc"] = _build_nc()
    return _CACHE["nc"]


def _to_mdt(a):
    if MM_DT == "bf16":
        import ml_dtypes
        return np.asarray(a, dtype=np.float32).astype(ml_dtypes.bfloat16)
    return np.asarray(a, dtype=np.float32)


def _run_on_hw(inputs, **spmd_kwargs):
    """Shard, run on the 8 NeuronCores, gather. Returns (out, BassKernelResults)."""
    from concourse.bass_utils import run_bass_kernel_spmd

    f = np.float32
    h = np.ascontiguousarray(np.asarray(inputs["h"], dtype=f))
    ws = {k: np.ascontiguousarray(np.asarray(inputs[k], dtype=f))
          for k in ("w_in", "b_in", "w_out", "b_out", "we1", "be1", "we2",
                    "be2", "wn1", "bn1", "wn2", "bn2")}
    nc = _get_nc()
    base = {
        "w_in": _to_mdt(ws["w_in"]), "b_in": ws["b_in"],
        "w_out": ws["w_out"], "b_out": ws["b_out"],
        "we1": _to_mdt(ws["we1"]), "be1": ws["be1"],
        "we2": _to_mdt(ws["we2"]), "be2": ws["be2"],
        "wn1": _to_mdt(ws["wn1"]), "bn1": ws["bn1"],
        "wn2": _to_mdt(ws["wn2"]), "bn2": ws["bn2"],
        "ident": _to_mdt(np.eye(128, dtype=f)),
        "identf": np.eye(128, dtype=f),
        "ones_r": np.ones((1, 128), dtype=f),
    }
    in_maps = []
    for c in range(NCORES):
        m = dict(base)
        m["h_c"] = np.ascontiguousarray(
            h[c * GPC:(c + 1) * GPC].reshape(NODES, IN_NF))
        in_maps.append(m)

    res = run_bass_kernel_spmd(nc, in_maps, list(range(NCORES)), **spmd_kwargs)
    out = np.concatenate([np.asarray(res.results[i]["out_c"], dtype=f)
                          for i in range(NCORES)], axis=0)
    return out, res


def kernel(**inputs):
    h = np.asarray(inputs["h"])
    rows, cols = inputs["rows"], inputs["cols"]
    if h.shape != (BS, N, IN_NF) or not _edges_match(rows, cols):
        ws = {k: np.asarray(inputs[k], dtype=np.float32)
              for k in ("w_in", "b_in", "w_out", "b_out", "we1", "be1", "we2",
                        "be2", "wn1", "bn1", "wn2", "bn2")}
        return _numpy_reference(np.asarray(h, np.float32), rows, cols, **ws)
    out, _ = _run_on_hw(inputs)
    return out

